# revision 12
# baseline (speedup 1.0000x reference)
"""DeStationaryAttention Trainium2 kernel (v2 — transpose-free).

Full inputs in, full output out. Sharding: B*N = 64 attention heads are
split across 8 NeuronCores, 8 heads each: core c handles batch b = c//2,
nodes n0 = (c%2)*8 .. n0+8.

Host-side prep is pure layout (slice / transpose / dtype-pack):
  QK  [H, 2, 128, 1024] f32  — Q^T and K^T per head ([d, t] major)
  V   [H, 128, 8, 128]  bf16 — V tiled [t%128, t//128, d]
  S   [1024, H] f32 (std), TW/TB [1, 1] — tau Linear params
Device returns OT [H, 128, 1024] f32 — the normalized attention output
transposed ([d, t]); host transposes back to [t, d].

Per-head math (T=1024, D=128):
  tau_eff = 2*sigmoid(mean_T(std)*w + b) * D^-0.5       (device prologue)
  qct     = (qT - mean_T(q)) * tau_eff                  (one DVE tensor_scalar)
  per s-tile i (8 of 128 rows):
    S^T_i = kT_i.T @ qct          (PE fp32r, 2 matmuls N=512)
    E^T_i = exp(S^T_i)            (ScalarE, PSUM->SBUF, bf16 out)
    O^T  += V_i.T @ E^T_i         (PE bf16, accumulated in PSUM)
  esum  = pairwise-tree sum of E tiles                  (DVE bf16 2x mode)
  rsbc  = ones128.T @ esum  — row sums broadcast to all partitions (PE)
  out   = O^T * reciprocal(rsbc)                        (DVE, evacuates PSUM)
K-centering is dropped: softmax_s(Qc·(K-muK)) == softmax_s(Qc·K).

Emission is software-pipelined across heads: DMA prefetch 2 heads ahead,
centering for head h+1 and finalize for head h-1 are interleaved into
head h's s-loop, and head h+1's first S/exp is emitted before head h's
tail AV matmuls so ScalarE never idles at head boundaries.
"""

import os
import sys
from contextlib import ExitStack

for _p in ("/root/.axon_site/_ro/trn_rl_repo", "/opt/trn_rl_repo"):
    if os.path.isdir(_p) and _p not in sys.path:
        sys.path.append(_p)

import numpy as np
import ml_dtypes

import concourse.bass as bass
import concourse.mybir as mybir
import concourse.tile as tile
from concourse import bacc
from concourse.bass_utils import run_bass_kernel_spmd

B, T, N, D = 4, 1024, 16, 128
H = 8           # heads per core
NCORES = 8
TT = T // 128   # 128-row tiles along T
F32 = mybir.dt.float32
F32R = mybir.dt.float32r
BF16 = mybir.dt.bfloat16
SCALE2 = 2.0 * D ** (-0.5)   # folds the 2*sigmoid(...) and D^-0.5 scales


def _r(ap):
    return ap.bitcast(F32R)


def _emit(tc):
    nc = tc.nc
    qk_d = nc.dram_tensor("QK", [H, D, 2, T], F32, kind="ExternalInput").ap()
    v_d = nc.dram_tensor("V", [H, 128, TT, D], BF16, kind="ExternalInput").ap()
    std_d = nc.dram_tensor("S", [T, H], F32, kind="ExternalInput").ap()
    tw_d = nc.dram_tensor("TW", [1, 1], F32, kind="ExternalInput").ap()
    tb_d = nc.dram_tensor("TB", [1, 1], F32, kind="ExternalInput").ap()
    o_d = nc.dram_tensor("O", [H, D, T], F32, kind="ExternalOutput").ap()

    Exp = mybir.ActivationFunctionType.Exp
    X = mybir.AxisListType.X
    Add = mybir.AluOpType.add
    Mult = mybir.AluOpType.mult

    ctx = ExitStack()
    const = ctx.enter_context(tc.tile_pool(name="const", bufs=1))
    qkp = ctx.enter_context(tc.tile_pool(name="qkp", bufs=3))
    etp = ctx.enter_context(tc.tile_pool(name="etp", bufs=6))
    trp = ctx.enter_context(tc.tile_pool(name="trp", bufs=3))
    finp = ctx.enter_context(tc.tile_pool(name="finp", bufs=2))
    smallp = ctx.enter_context(tc.tile_pool(name="smallp", bufs=2))
    taup = ctx.enter_context(tc.tile_pool(name="taup", bufs=H))
    ps = ctx.enter_context(tc.tile_pool(name="ps", bufs=2, space="PSUM"))

    # ---- constants ----
    ones_bf = const.tile([128, 128], BF16)
    nc.vector.memset(ones_bf, 1.0)
    inv_t = const.tile([128, 1], F32)
    nc.vector.memset(inv_t, 1.0 / T)
    bc2 = const.tile([1, 128], F32)
    nc.vector.memset(bc2, SCALE2)
    std_sb = const.tile([128, T * H // 128], F32)   # [128, 64]
    nc.sync.dma_start(out=std_sb, in_=std_d.rearrange("(p j) h -> p (j h)", p=128))
    tw_sb = const.tile([1, 1], F32)
    nc.sync.dma_start(out=tw_sb, in_=tw_d)
    tb_sb = const.tile([1, 1], F32)
    nc.sync.dma_start(out=tb_sb, in_=tb_d)
    negw = const.tile([1, 1], F32)
    nc.vector.tensor_scalar_mul(negw, tw_sb, -1.0)
    negb = const.tile([1, 1], F32)
    nc.vector.tensor_scalar_mul(negb, tb_sb, -1.0)
    std3 = std_sb.rearrange("p (j h) -> p j h", h=H)

    # ---- per-head tau_eff = 2*sigmoid(mean(std)*w+b)*D^-0.5, as [128,1] ----
    tau_scs = []
    for h in range(H):
        part = smallp.tile([128, 1], F32, tag="part")
        nc.vector.reduce_sum(out=part, in_=std3[:, :, h], axis=X)
        mean_ps = ps.tile([1, 1], F32, tag="st")
        nc.tensor.matmul(mean_ps, lhsT=inv_t, rhs=part, start=True, stop=True)
        ez = smallp.tile([1, 1], F32, tag="ez")
        nc.scalar.activation(ez, mean_ps, Exp, bias=negb[:], scale=negw[:])
        den = smallp.tile([1, 1], F32, tag="den")
        nc.vector.tensor_scalar_add(den, ez, 1.0)
        sig = smallp.tile([1, 1], F32, tag="sig")
        nc.vector.reciprocal(sig, den)
        tau_ps = ps.tile([128, 1], F32, tag="st")
        nc.tensor.matmul(tau_ps, lhsT=bc2, rhs=sig, start=True, stop=True)
        tau_sc = taup.tile([128, 1], F32, tag="tau_sc")
        nc.vector.tensor_copy(tau_sc, tau_ps)
        tau_scs.append(tau_sc)

    states = [dict() for _ in range(H)]

    def prep_dma(h):
        st = states[h]
        qk = qkp.tile([128, 2 * T], F32R, tag="qk", name="qk")
        nc.sync.dma_start(out=qk, in_=_r(qk_d[h].rearrange("d two t -> d (two t)")))
        v = qkp.tile([128, TT, 128], BF16, tag="v", name="v")
        nc.sync.dma_start(out=v, in_=v_d[h])
        st["qk"], st["v"] = qk, v

    def prep_center(h):
        # DVE: column means of qT, then qct = (qT - mu) * tau_eff
        st = states[h]
        qT = st["qk"][:, 0:T].bitcast(F32)
        qsum = smallp.tile([128, 1], F32, tag="qsum")
        nc.vector.reduce_sum(out=qsum, in_=qT, axis=X)
        negmu = smallp.tile([128, 1], F32, tag="negmu")
        nc.vector.tensor_scalar_mul(negmu, qsum, -1.0 / T)
        qct = qkp.tile([128, T], F32R, tag="qct", name="qct")
        nc.vector.tensor_scalar(qct, qT, negmu[:], tau_scs[h][:], op0=Add, op1=Mult)
        st["qct"] = qct

    def emit_av(st, i, et_slice):
        ot, v = st["ot"], st["v"]
        vl = v[:, i, :]
        nc.tensor.matmul(ot[:, 0:512], lhsT=vl, rhs=et_slice[:, 0:512],
                         start=(i == 0), stop=(i == TT - 1))
        nc.tensor.matmul(ot[:, 512:1024], lhsT=vl, rhs=et_slice[:, 512:1024],
                         start=(i == 0), stop=(i == TT - 1))

    def fin_pe(h):
        # row sums of E broadcast to all partitions: rsbc = ones128.T @ esum,
        # accumulating the two halves of tree-C so no DVE fold is needed
        st = states[h]
        e2 = st["esum2"]
        rsbc = ps.tile([128, T], F32, tag="st", name="rsbc")
        for half in range(2):
            s, e = (half == 0), (half == 1)
            nc.tensor.matmul(rsbc[:, 0:512], lhsT=ones_bf,
                             rhs=e2[:, half * T:half * T + 512], start=s, stop=e)
            nc.tensor.matmul(rsbc[:, 512:1024], lhsT=ones_bf,
                             rhs=e2[:, half * T + 512:half * T + 1024], start=s, stop=e)
        st["rsbc"] = rsbc

    def fin_dve(h):
        # out = O^T * (1/rowsum); both ops evacuate PSUM as they go
        st = states[h]
        rcp = finp.tile([128, T], F32, tag="rcp", name="rcp")
        nc.vector.reciprocal_approx_fast(rcp, st["rsbc"])
        ots = finp.tile([128, T], F32, tag="ots", name="ots")
        nc.vector.tensor_mul(ots, st["ot"], rcp)
        nc.gpsimd.dma_start(out=o_d[h], in_=ots)

    def sloop(h, lo, hi):
        st = states[h]
        qct = st["qct"]
        kT = st["qk"][:, T:2 * T]
        if lo == 0:
            st["ot"] = ps.tile([128, T], F32, tag="ot", name="ot")
            st["pairs"] = []
            st["pend"] = []
        pairs, pend = st["pairs"], st["pend"]
        for i in range(lo, hi):
            if i == 1 and h + 2 < H:
                prep_dma(h + 2)
            if i == 3 and h > 0:
                fin_pe(h - 1)
            if i == 4 and h > 0:
                fin_dve(h - 1)
            if i == 4:
                treeA = trp.tile([128, 2 * T], BF16, tag="tree", name="treeA")
                nc.vector.tensor_add(treeA, pairs[0], pairs[1])
                st["treeA"] = treeA
            if i == 5 and h + 1 < H:
                prep_center(h + 1)
            klhs = kT[:, i * 128:(i + 1) * 128]
            stp = ps.tile([128, T], F32, tag="st", name="stp")
            nc.tensor.matmul(stp[:, 0:512], lhsT=klhs, rhs=qct[:, 0:512],
                             start=True, stop=True)
            nc.tensor.matmul(stp[:, 512:1024], lhsT=klhs, rhs=qct[:, 512:1024],
                             start=True, stop=True)
            if i % 2 == 0:
                etpair = etp.tile([128, 2 * T], BF16, tag="et", name="etpair")
                pairs.append(etpair)
            etpair = pairs[-1]
            z = (i % 2) * T
            nc.scalar.activation(etpair[:, z:z + T], stp, Exp)
            pend.append((i, etpair[:, z:z + T]))
            if len(pend) > 2:
                emit_av(st, *pend.pop(0))

    def sloop_tail(h):
        # drain AVs, then the esum tree: B = p2+p3, C = A+B, esum = fold(C)
        st = states[h]
        pairs, pend = st["pairs"], st["pend"]
        while pend:
            emit_av(st, *pend.pop(0))
        treeB = trp.tile([128, 2 * T], BF16, tag="tree", name="treeB")
        nc.vector.tensor_add(treeB, pairs[2], pairs[3])
        treeC = trp.tile([128, 2 * T], BF16, tag="tree", name="treeC")
        nc.vector.tensor_add(treeC, st["treeA"], treeB)
        st["esum2"] = treeC

    # ---- software-pipelined emission ----
    prep_dma(0)
    prep_dma(1)
    prep_center(0)
    sloop(0, 0, TT)
    for h in range(1, H):
        # head h's first S/exp lands before head h-1's tail AVs so the
        # ScalarE exp stream never gaps at the head boundary
        sloop(h, 0, 1)
        sloop_tail(h - 1)
        sloop(h, 1, TT)
    sloop_tail(H - 1)
    fin_pe(H - 1)
    fin_dve(H - 1)
    ctx.close()


_BUILT = None


def _build():
    global _BUILT
    if _BUILT is None:
        nc = bacc.Bacc("TRN2", target_bir_lowering=False, debug=False, num_devices=None)
        with tile.TileContext(nc) as tc:
            _emit(tc)
        nc.compile()
        _BUILT = nc
    return _BUILT


def _in_maps(Q, K, V, std, tau_w, tau_b):
    tw = np.asarray(tau_w, np.float32).reshape(1, 1)
    tb = np.asarray(tau_b, np.float32).reshape(1, 1)
    maps = []
    for c in range(NCORES):
        b, n0 = c // 2, (c % 2) * H
        # [T, H, D] -> [H, D, T] transposed views, packed [H, D, 2, T]
        qk = np.empty((H, D, 2, T), np.float32)
        qk[:, :, 0] = Q[b, :, n0:n0 + H, :].transpose(1, 2, 0)
        qk[:, :, 1] = K[b, :, n0:n0 + H, :].transpose(1, 2, 0)
        # V: [T, H, D] -> [H, t%128, t//128, D] in bf16
        v = np.ascontiguousarray(
            V[b, :, n0:n0 + H, :].reshape(TT, 128, H, D).transpose(2, 1, 0, 3)
        ).astype(ml_dtypes.bfloat16)
        maps.append({
            "QK": qk,
            "V": v,
            "S": np.ascontiguousarray(std[b, :, n0:n0 + H, 0], np.float32),
            "TW": tw,
            "TB": tb,
        })
    return maps


def _gather(results):
    out = np.empty((B, T, N, D), np.float32)
    for c in range(NCORES):
        b, n0 = c // 2, (c % 2) * H
        # OT [H, D, T] -> [T, H, D]
        out[b, :, n0:n0 + H, :] = results[c]["O"].transpose(2, 0, 1)
    return out


def run(Q, K, V, std, tau_w, tau_b, **spmd_kwargs):
    nc = _build()
    res = run_bass_kernel_spmd(nc, _in_maps(Q, K, V, std, tau_w, tau_b),
                               core_ids=list(range(NCORES)), **spmd_kwargs)
    return _gather(res.results), res


def kernel(Q, K, V, std, tau_w, tau_b):
    out, _ = run(Q, K, V, std, tau_w, tau_b)
    return out


# revision 13
# speedup vs baseline: 1.0852x; 1.0852x over previous
"""DeStationaryAttention Trainium2 kernel (v2 — transpose-free).

Full inputs in, full output out. Sharding: B*N = 64 attention heads are
split across 8 NeuronCores, 8 heads each: core c handles batch b = c//2,
nodes n0 = (c%2)*8 .. n0+8.

Host-side prep is pure layout (slice / transpose / dtype-pack):
  QK  [H, 2, 128, 1024] f32  — Q^T and K^T per head ([d, t] major)
  V   [H, 128, 8, 128]  bf16 — V tiled [t%128, t//128, d]
  S   [1024, H] f32 (std), TW/TB [1, 1] — tau Linear params
Device returns OT [H, 128, 1024] f32 — the normalized attention output
transposed ([d, t]); host transposes back to [t, d].

Per-head math (T=1024, D=128):
  tau_eff = 2*sigmoid(mean_T(std)*w + b) * D^-0.5       (device prologue)
  qct     = (qT - mean_T(q)) * tau_eff                  (one DVE tensor_scalar)
  per s-tile i (8 of 128 rows):
    S^T_i = kT_i.T @ qct          (PE fp32r, 2 matmuls N=512)
    E^T_i = exp(S^T_i)            (ScalarE, PSUM->SBUF, bf16 out)
    O^T  += V_i.T @ E^T_i         (PE bf16, accumulated in PSUM)
  esum  = pairwise-tree sum of E tiles                  (DVE bf16 2x mode)
  rsbc  = ones128.T @ esum  — row sums broadcast to all partitions (PE)
  out   = O^T * reciprocal(rsbc)                        (DVE, evacuates PSUM)
K-centering is dropped: softmax_s(Qc·(K-muK)) == softmax_s(Qc·K).

Emission is software-pipelined across heads: DMA prefetch 2 heads ahead,
centering for head h+1 and finalize for head h-1 are interleaved into
head h's s-loop, and head h+1's first S/exp is emitted before head h's
tail AV matmuls so ScalarE never idles at head boundaries.
"""

import os
import sys
from contextlib import ExitStack

for _p in ("/root/.axon_site/_ro/trn_rl_repo", "/opt/trn_rl_repo"):
    if os.path.isdir(_p) and _p not in sys.path:
        sys.path.append(_p)

import numpy as np
import ml_dtypes

import concourse.bass as bass
import concourse.mybir as mybir
import concourse.tile as tile
from concourse import bacc
from concourse.bass_utils import run_bass_kernel_spmd

B, T, N, D = 4, 1024, 16, 128
H = 8           # heads per core
NCORES = 8
TT = T // 128   # 128-row tiles along T
F32 = mybir.dt.float32
F32R = mybir.dt.float32r
BF16 = mybir.dt.bfloat16
SCALE2 = 2.0 * D ** (-0.5)   # folds the 2*sigmoid(...) and D^-0.5 scales


def _r(ap):
    return ap.bitcast(F32R)


def _emit(tc):
    nc = tc.nc
    qk_d = nc.dram_tensor("QK", [H, D, 2, T], F32, kind="ExternalInput").ap()
    v_d = nc.dram_tensor("V", [H, 128, TT, D], BF16, kind="ExternalInput").ap()
    std_d = nc.dram_tensor("S", [T, H], F32, kind="ExternalInput").ap()
    tw_d = nc.dram_tensor("TW", [1, 1], F32, kind="ExternalInput").ap()
    tb_d = nc.dram_tensor("TB", [1, 1], F32, kind="ExternalInput").ap()
    o_d = nc.dram_tensor("O", [H, D, T], F32, kind="ExternalOutput").ap()

    Exp = mybir.ActivationFunctionType.Exp
    X = mybir.AxisListType.X
    Add = mybir.AluOpType.add
    Mult = mybir.AluOpType.mult

    ctx = ExitStack()
    const = ctx.enter_context(tc.tile_pool(name="const", bufs=1))
    qkp = ctx.enter_context(tc.tile_pool(name="qkp", bufs=3))
    etp = ctx.enter_context(tc.tile_pool(name="etp", bufs=6))
    trp = ctx.enter_context(tc.tile_pool(name="trp", bufs=3))
    finp = ctx.enter_context(tc.tile_pool(name="finp", bufs=2))
    smallp = ctx.enter_context(tc.tile_pool(name="smallp", bufs=2))
    taup = ctx.enter_context(tc.tile_pool(name="taup", bufs=H))
    ps = ctx.enter_context(tc.tile_pool(name="ps", bufs=2, space="PSUM"))

    # ---- constants ----
    ones_bf = const.tile([128, 128], BF16)
    nc.vector.memset(ones_bf, 1.0)
    inv_t = const.tile([128, 1], F32)
    nc.vector.memset(inv_t, 1.0 / T)
    bc2 = const.tile([1, 128], F32)
    nc.vector.memset(bc2, SCALE2)
    states = [dict() for _ in range(H)]
    tau_scs = []

    def prep_dma(h):
        st = states[h]
        qk = qkp.tile([128, 2 * T], F32R, tag="qk", name="qk")
        nc.sync.dma_start(out=qk, in_=_r(qk_d[h].rearrange("d two t -> d (two t)")))
        v = qkp.tile([128, TT, 128], BF16, tag="v", name="v")
        nc.sync.dma_start(out=v, in_=v_d[h])
        st["qk"], st["v"] = qk, v

    def prep_center(h):
        # DVE: column means of qT, then qct = (qT - mu) * tau_eff
        st = states[h]
        qT = st["qk"][:, 0:T].bitcast(F32)
        qsum = smallp.tile([128, 1], F32, tag="qsum")
        nc.vector.reduce_sum(out=qsum, in_=qT, axis=X)
        negmu = smallp.tile([128, 1], F32, tag="negmu")
        nc.vector.tensor_scalar_mul(negmu, qsum, -1.0 / T)
        qct = qkp.tile([128, T], F32R, tag="qct", name="qct")
        nc.vector.tensor_scalar(qct, qT, negmu[:], tau_scs[h][:], op0=Add, op1=Mult)
        st["qct"] = qct

    def emit_av(st, i, et_slice):
        ot, v = st["ot"], st["v"]
        vl = v[:, i, :]
        nc.tensor.matmul(ot[:, 0:512], lhsT=vl, rhs=et_slice[:, 0:512],
                         start=(i == 0), stop=(i == TT - 1))
        nc.tensor.matmul(ot[:, 512:1024], lhsT=vl, rhs=et_slice[:, 512:1024],
                         start=(i == 0), stop=(i == TT - 1))

    def fin_pe(h):
        # row sums of E broadcast to all partitions: rsbc = ones128.T @ esum,
        # accumulating the two halves of tree-C so no DVE fold is needed
        st = states[h]
        e2 = st["esum2"]
        rsbc = ps.tile([128, T], F32, tag="st", name="rsbc")
        for half in range(2):
            s, e = (half == 0), (half == 1)
            nc.tensor.matmul(rsbc[:, 0:512], lhsT=ones_bf,
                             rhs=e2[:, half * T:half * T + 512], start=s, stop=e)
            nc.tensor.matmul(rsbc[:, 512:1024], lhsT=ones_bf,
                             rhs=e2[:, half * T + 512:half * T + 1024], start=s, stop=e)
        st["rsbc"] = rsbc

    def fin_dve(h):
        # out = O^T * (1/rowsum); both ops evacuate PSUM as they go
        st = states[h]
        rcp = finp.tile([128, T], F32, tag="rcp", name="rcp")
        nc.vector.reciprocal_approx_fast(rcp, st["rsbc"])
        ots = finp.tile([128, T], F32, tag="ots", name="ots")
        nc.vector.tensor_mul(ots, st["ot"], rcp)
        nc.gpsimd.dma_start(out=o_d[h], in_=ots)

    def sloop(h, lo, hi):
        st = states[h]
        qct = st["qct"]
        kT = st["qk"][:, T:2 * T]
        if lo == 0:
            st["ot"] = ps.tile([128, T], F32, tag="ot", name="ot")
            st["pairs"] = []
            st["pend"] = []
        pairs, pend = st["pairs"], st["pend"]
        for i in range(lo, hi):
            if i == 1 and h + 2 < H:
                prep_dma(h + 2)
            if i == 3 and h > 0:
                fin_pe(h - 1)
            if i == 4 and h > 0:
                fin_dve(h - 1)
            if i == 4:
                treeA = trp.tile([128, 2 * T], BF16, tag="tree", name="treeA")
                nc.vector.tensor_add(treeA, pairs[0], pairs[1])
                st["treeA"] = treeA
            if i == 5 and h == H - 1:
                # last head: accumulate rowsums from treeA now (tail shortening)
                rsbc = ps.tile([128, T], F32, tag="ot", name="rsbc7")
                st["rsbc"] = rsbc
                for half in range(2):
                    z = half * 512
                    nc.tensor.matmul(rsbc[:, z:z + 512], lhsT=ones_bf,
                                     rhs=st["treeA"][:, z:z + 512],
                                     start=True, stop=False)
                    nc.tensor.matmul(rsbc[:, z:z + 512], lhsT=ones_bf,
                                     rhs=st["treeA"][:, T + z:T + z + 512],
                                     start=False, stop=False)
            if i == 5 and h + 1 < H:
                prep_center(h + 1)
            klhs = kT[:, i * 128:(i + 1) * 128]
            stp = ps.tile([128, T], F32, tag="st", name="stp")
            nc.tensor.matmul(stp[:, 0:512], lhsT=klhs, rhs=qct[:, 0:512],
                             start=True, stop=True)
            nc.tensor.matmul(stp[:, 512:1024], lhsT=klhs, rhs=qct[:, 512:1024],
                             start=True, stop=True)
            if i % 2 == 0:
                etpair = etp.tile([128, 2 * T], BF16, tag="et", name="etpair")
                pairs.append(etpair)
            etpair = pairs[-1]
            z = (i % 2) * T
            nc.scalar.activation(etpair[:, z:z + T], stp, Exp)
            pend.append((i, etpair[:, z:z + T]))
            if len(pend) > 2:
                emit_av(st, *pend.pop(0))

    def sloop_tail(h):
        # drain AVs, then the esum tree: B = p2+p3, C = A+B, esum = fold(C)
        st = states[h]
        pairs, pend = st["pairs"], st["pend"]
        while pend:
            emit_av(st, *pend.pop(0))
        treeB = trp.tile([128, 2 * T], BF16, tag="tree", name="treeB")
        nc.vector.tensor_add(treeB, pairs[2], pairs[3])
        if h == H - 1:
            rsbc = st["rsbc"]
            for half in range(2):
                z = half * 512
                nc.tensor.matmul(rsbc[:, z:z + 512], lhsT=ones_bf,
                                 rhs=treeB[:, z:z + 512], start=False, stop=False)
                nc.tensor.matmul(rsbc[:, z:z + 512], lhsT=ones_bf,
                                 rhs=treeB[:, T + z:T + z + 512],
                                 start=False, stop=True)
        else:
            treeC = trp.tile([128, 2 * T], BF16, tag="tree", name="treeC")
            nc.vector.tensor_add(treeC, st["treeA"], treeB)
            st["esum2"] = treeC

    # ---- software-pipelined emission ----
    prep_dma(0)
    std_sb = const.tile([128, T * H // 128], F32)   # [128, 64]
    nc.sync.dma_start(out=std_sb, in_=std_d.rearrange("(p j) h -> p (j h)", p=128))
    tw_sb = const.tile([1, 1], F32)
    nc.sync.dma_start(out=tw_sb, in_=tw_d)
    tb_sb = const.tile([1, 1], F32)
    nc.sync.dma_start(out=tb_sb, in_=tb_d)
    prep_dma(1)
    # batched tau prologue: all 8 heads in one [128,8]/[1,8] chain
    negw = const.tile([1, 1], F32)
    nc.vector.tensor_scalar_mul(negw, tw_sb, -1.0)
    negb = const.tile([1, 1], F32)
    nc.vector.tensor_scalar_mul(negb, tb_sb, -1.0)
    part8 = const.tile([128, H], F32)
    nc.vector.reduce_sum(out=part8, in_=std_sb.rearrange("p (j h) -> p h j", h=H),
                         axis=X)
    mean8_ps = ps.tile([1, H], F32, tag="st")
    nc.tensor.matmul(mean8_ps, lhsT=inv_t, rhs=part8, start=True, stop=True)
    ez8 = const.tile([1, H], F32)
    nc.scalar.activation(ez8, mean8_ps, Exp, bias=negb[:], scale=negw[:])
    den8 = const.tile([1, H], F32)
    nc.vector.tensor_scalar_add(den8, ez8, 1.0)
    sig8 = const.tile([1, H], F32)
    nc.vector.reciprocal(sig8, den8)
    tau8_ps = ps.tile([128, H], F32, tag="st")
    nc.tensor.matmul(tau8_ps, lhsT=bc2, rhs=sig8, start=True, stop=True)
    tau8 = const.tile([128, H], F32)
    nc.vector.tensor_copy(tau8, tau8_ps)
    for h in range(H):
        tau_scs.append(tau8[:, h:h + 1])
    prep_center(0)
    sloop(0, 0, TT)
    for h in range(1, H):
        # head h's first S/exp lands before head h-1's tail AVs so the
        # ScalarE exp stream never gaps at the head boundary
        sloop(h, 0, 1)
        sloop_tail(h - 1)
        sloop(h, 1, TT)
    sloop_tail(H - 1)
    fin_dve(H - 1)
    ctx.close()


_BUILT = None


def _build():
    global _BUILT
    if _BUILT is None:
        nc = bacc.Bacc("TRN2", target_bir_lowering=False, debug=False, num_devices=None)
        with tile.TileContext(nc) as tc:
            _emit(tc)
        nc.compile()
        _BUILT = nc
    return _BUILT


def _in_maps(Q, K, V, std, tau_w, tau_b):
    tw = np.asarray(tau_w, np.float32).reshape(1, 1)
    tb = np.asarray(tau_b, np.float32).reshape(1, 1)
    maps = []
    for c in range(NCORES):
        b, n0 = c // 2, (c % 2) * H
        # [T, H, D] -> [H, D, T] transposed views, packed [H, D, 2, T]
        qk = np.empty((H, D, 2, T), np.float32)
        qk[:, :, 0] = Q[b, :, n0:n0 + H, :].transpose(1, 2, 0)
        qk[:, :, 1] = K[b, :, n0:n0 + H, :].transpose(1, 2, 0)
        # V: [T, H, D] -> [H, t%128, t//128, D] in bf16
        v = np.ascontiguousarray(
            V[b, :, n0:n0 + H, :].reshape(TT, 128, H, D).transpose(2, 1, 0, 3)
        ).astype(ml_dtypes.bfloat16)
        maps.append({
            "QK": qk,
            "V": v,
            "S": np.ascontiguousarray(std[b, :, n0:n0 + H, 0], np.float32),
            "TW": tw,
            "TB": tb,
        })
    return maps


def _gather(results):
    out = np.empty((B, T, N, D), np.float32)
    for c in range(NCORES):
        b, n0 = c // 2, (c % 2) * H
        # OT [H, D, T] -> [T, H, D]
        out[b, :, n0:n0 + H, :] = results[c]["O"].transpose(2, 0, 1)
    return out


def run(Q, K, V, std, tau_w, tau_b, **spmd_kwargs):
    nc = _build()
    res = run_bass_kernel_spmd(nc, _in_maps(Q, K, V, std, tau_w, tau_b),
                               core_ids=list(range(NCORES)), **spmd_kwargs)
    return _gather(res.results), res


def kernel(Q, K, V, std, tau_w, tau_b):
    out, _ = run(Q, K, V, std, tau_w, tau_b)
    return out
